# revision 8
# baseline (speedup 1.0000x reference)
"""LIF spike recurrence kernel for Trainium2 (8 NeuronCores, SPMD). v11.

Problem: x [32, 128, 32, 32, 8] f32, recurrence over last (time) dim:
    u_t = TAU * u_{t-1} * (1 - o_{t-1}) + x_t
    o_t = 1[u_t - VTH > 0]
Output: o [32, 128, 32, 32, 8] f32 (0.0 / 1.0 spikes).

Design (v6 was DVE-bound at 81.6us: 14 full-plane STT ops = 62us serial on
DVE while the input stream lands by 49us exec. Probed alternatives: Pool
ALU is slow Q7 software that also poisons DVE SBUF bandwidth; PE fp32
matmul is ~6 cycles/col and contends for SBUF. The winning second ALU is
the DMA itself: SWDGE (gpsimd-issued) DMA supports accum_op=add, probed
BIT-EXACT fp32 at ~190 GB/s for chunks <= 8KB/partition):
  - Shard batch (32) across 8 cores -> 4/core; host pre-transposes each
    shard to plane-major [P=128, T=8, NPP=4096] and PRE-SCALES plane t by
    4^t (exact), switching to w-space: w_t = u_t * 4^t obeys
        w_t = w_{t-1} * [w_{t-1} <= VTH*4^(t-1)] + x'_t,   x'_t = x_t*4^t
    removing the TAU multiply (power-of-two scaling commutes with fp32
    rounding -> bit-exact vs the reference).
  - Columns [0, A): ACCUM chain. DVE writes c = (w<=TH)*w directly into
    the xt[:, t, :] plane slot (STT); the plane-t input DMA then rides the
    SWDGE queue with accum_op=add, so the DMA engines compute
    w_t = c + x'_t while streaming the input. Two sub-chunks per plane
    hide the transfer latency inside the DVE's step time.
  - Columns [A, NPP): classic SBUF chain: STT -> c tile, TT-add in place
    over the sync-ring-streamed x'.
    DVE per step: NPP + (NPP-A) columns instead of 2*NPP -> 62us -> 46us,
    with the accum queue (190 GB/s) absorbing adds for A columns.
  - Spike via ScalarE: o8 = Sign(w * 4^-t - VTH) -> int8 full plane
    (scale is an exact power of two = the reference's u_t - VTH compare);
    output DMA rides the Scalar HWDGE ring. Plane-7 signs/outs chunked.
  - Host maps >0 to 1.0f (exact). int8 output cuts out-DMA 4x vs f32.
"""

import numpy as np

TAU = 0.25
VTH = 0.3
N_CORES = 8
P = 128
T = 8
B_LOC = 4  # batches per core
PIX_PER_CORE = B_LOC * 128 * 32 * 32  # 524288
NPP = PIX_PER_CORE // P  # 4096 pixels per partition

_CACHE = {}

# Config key, A/B-tested on hardware. Fields:
#   a<j>   : accum-chain columns [0,j), balanced vs the 190 GB/s SWDGE rate
#   noaeb  : barrier only {Pool->Activation} instead of all-engine
CFG = "v11_a2304"


def _parse(key):
    a = 2304
    for tok in key.split("_"):
        if tok.startswith("a") and tok[1:].isdigit():
            a = int(tok[1:])
    return dict(a=a, noaeb="noaeb" in key)


def _isect(lo, hi, bounds):
    """Sub-ranges of [lo,hi) cut at the given ascending bounds."""
    cuts = sorted({lo, hi, *[b for b in bounds if lo < b < hi]})
    return list(zip(cuts[:-1], cuts[1:]))


def _th(t):
    """Spike threshold in w-space at step t (exact power-of-two scaling)."""
    return VTH * (4.0 ** t)


def _build_nc(key=None):
    if key is None:
        key = CFG
    cfg = _parse(key)
    import concourse.tile as tile
    from concourse import bacc, mybir

    f32 = mybir.dt.float32
    i8 = mybir.dt.int8
    Alu = mybir.AluOpType
    AF = mybir.ActivationFunctionType

    nc = bacc.Bacc(
        "TRN2",
        target_bir_lowering=False,
        debug=False,
        enable_asserts=False,
        num_devices=N_CORES,
    )
    x_d = nc.dram_tensor("x", [P, T, NPP], f32, kind="ExternalInput").ap()
    o_d = nc.dram_tensor("o", [P, T, NPP], i8, kind="ExternalOutput").ap()

    # ACT activation bias needs a pre-registered const AP.
    cb = nc.alloc_sbuf_tensor("const-f32-negvth", [128, 1], f32)
    nc.gpsimd.memset(cb.ap(), -VTH)
    nc.const_aps.aps[(f32, -VTH)] = cb.ap()
    if cfg["noaeb"]:
        nc.multi_engine_barrier(
            [mybir.EngineType.Pool, mybir.EngineType.Activation]
        )
    else:
        nc.all_engine_barrier()

    A = cfg["a"]                      # accum-chain column count
    AC = [0, A // 2, A]               # accum sub-chunks (<=2048 cols each)
    head_bounds = [0, 2048, NPP]      # plane-0 / plane-1-sync DMA chunks
    q7 = [A, 3072, NPP]               # plane-7 sync-part DMA chunks

    with tile.TileContext(nc) as tc:
        with tc.tile_pool(name="pp", bufs=1) as pp:
            xt = pp.tile([P, T, NPP], f32, tag="xt")
            c = pp.tile([P, NPP], f32, tag="c")
            o8 = pp.tile([P, T, NPP], i8, tag="o8")

            # --- sync-ring input DMAs (plane 0 whole; planes 1.. only the
            # SBUF-chain columns [A, NPP)). Plane 0/1 interleaved. ---
            p1_chunks = _isect(A, NPP, head_bounds)
            for i, (lo, hi) in enumerate(_isect(0, NPP, head_bounds)):
                nc.sync.dma_start(xt[:, 0, lo:hi], x_d[:, 0, lo:hi])
                if i < len(p1_chunks):
                    lo1, hi1 = p1_chunks[i]
                    nc.sync.dma_start(xt[:, 1, lo1:hi1], x_d[:, 1, lo1:hi1])
            for t in range(2, T - 1):
                nc.sync.dma_start(xt[:, t, A:], x_d[:, t, A:])
            for lo, hi in _isect(A, NPP, q7):
                nc.sync.dma_start(xt[:, 7, lo:hi], x_d[:, 7, lo:hi])

            def stt_acc(t, lo, hi):
                # c for accum cols goes straight into the plane-t slot; the
                # SWDGE accum DMA then adds x'_t on top of it.
                sl = slice(lo, hi)
                up = xt[:, t - 1, sl]
                nc.vector.scalar_tensor_tensor(
                    xt[:, t, sl], up, _th(t - 1), up, op0=Alu.is_le, op1=Alu.mult
                )
                nc.gpsimd.dma_start(
                    xt[:, t, sl], x_d[:, t, sl], accum_op=Alu.add
                )

            def cu_sbuf(t, lo, hi):
                sl = slice(lo, hi)
                up = xt[:, t - 1, sl]
                nc.vector.scalar_tensor_tensor(
                    c[:, sl], up, _th(t - 1), up, op0=Alu.is_le, op1=Alu.mult
                )
                nc.vector.tensor_tensor(
                    xt[:, t, sl], c[:, sl], xt[:, t, sl], op=Alu.add
                )

            def sign(t, lo, hi):
                sc = 0.25 ** t
                nc.scalar.activation(
                    o8[:, t, lo:hi], xt[:, t, lo:hi], AF.Sign,
                    bias=-VTH, scale=sc,
                )
                nc.scalar.dma_start(o_d[:, t, lo:hi], o8[:, t, lo:hi])

            # Plane 0: w_0 = x'_0.
            sign(0, 0, NPP)

            # Steps 1..6.
            for t in range(1, T - 1):
                for lo, hi in zip(AC[:-1], AC[1:]):
                    stt_acc(t, lo, hi)
                if t == 1:
                    for lo, hi in _isect(A, NPP, head_bounds):
                        cu_sbuf(t, lo, hi)
                else:
                    cu_sbuf(t, A, NPP)
                sign(t, 0, NPP)

            # Step 7: chunked signs/outs for a short tail.
            t = T - 1
            for lo, hi in zip(AC[:-1], AC[1:]):
                stt_acc(t, lo, hi)
                sign(t, lo, hi)
            for lo, hi in _isect(A, NPP, q7):
                cu_sbuf(t, lo, hi)
                sign(t, lo, hi)
    nc.compile()
    return nc


def _get_nc(key=None):
    k = key or CFG
    if k not in _CACHE:
        _CACHE[k] = _build_nc(k)
    return _CACHE[k]


_WSCALE = (4.0 ** np.arange(T)).astype(np.float32)  # exact powers of two


def _shard(x: np.ndarray):
    xs = np.ascontiguousarray(x, dtype=np.float32) * _WSCALE  # w-space
    return [
        np.ascontiguousarray(
            xs[i * B_LOC : (i + 1) * B_LOC].reshape(P, NPP, T).transpose(0, 2, 1)
        )
        for i in range(N_CORES)
    ]


def _run(in_maps, key=None, **kwargs):
    from concourse.bass_utils import run_bass_kernel_spmd

    nc = _get_nc(key)
    return run_bass_kernel_spmd(nc, in_maps, core_ids=list(range(N_CORES)), **kwargs)


def kernel(x: np.ndarray) -> np.ndarray:
    in_maps = [{"x": s} for s in _shard(x)]
    res = _run(in_maps)
    outs = []
    for i in range(N_CORES):
        s8 = res.results[i]["o"]  # [P, T, NPP] int8 sign values
        o = (s8 > 0).transpose(0, 2, 1).astype(np.float32)  # [P, NPP, T]
        outs.append(o.reshape(B_LOC, 128, 32, 32, T))
    return np.concatenate(outs, axis=0)


# revision 13
# speedup vs baseline: 1.3107x; 1.3107x over previous
"""LIF spike recurrence kernel for Trainium2 (8 NeuronCores, SPMD). v12.

Problem: x [32, 128, 32, 32, 8] f32, recurrence over last (time) dim:
    u_t = TAU * u_{t-1} * (1 - o_{t-1}) + x_t
    o_t = 1[u_t - VTH > 0]
Output: o [32, 128, 32, 32, 8] f32 (0.0 / 1.0 spikes).

Design (all facts hardware-probed):
  - Shard batch (32) across 8 cores -> 4/core; host pre-transposes each shard
    to plane-major [P=128, T=8, NPP=4096] so every SBUF access is contiguous
    (strided fp32 STT costs ~1.6x; contiguous runs at (FD+151)/0.96 ns exact,
    back-to-back with ~40ns gaps).
  - One mega-tile [P, T, NPP] per core; input DMA chunked (planes 0/1 in
    interleaved column chunks, so step-1 compute starts ~4 us earlier).
  - Exact fp32 recurrence on DVE, bit-identical to the reference:
       c   = (u_{t-1} <= VTH) * u_{t-1}     (STT is_le/mult; x{0,1} exact)
       u_t = c * TAU + x_t   in place       (TAU=2^-2 exact; single round)
  - Spike via ScalarE: o8_t = Sign(u_t - VTH) -> int8 {-1,0,1}, one ACTIVATE
    per plane ((FD+352)/1.2 ns, no bubble), fully hidden under DVE. Host maps
    >0 to 1.0f (exact). int8 output cuts out-DMA 4x vs f32.
  - Output DMA rides the Scalar HWDGE ring (input uses Sync's) so the queues
    never serialize.
  - v12 over v6: the DVE chain (62us serial) outruns the one-ring input
    stream only at the start; early steps stalled ~2-3us waiting for plane
    arrivals, and step 1 started at ~15us. Now planes 0/1 interleave in
    1K-column chunks (step 1 starts ~2.6us earlier) and the second halves
    of planes 2-4 ride the Scalar HWDGE ring (enqueued before any output
    DMA, so they clear the ring first), which pulls plane arrivals ahead of
    the DVE chain for the whole run.
  - Step 1 is column-chunked and step 7 column-quartered (signs/outs
    interleaved) to shorten the pipeline head and tail.
"""

import numpy as np

TAU = 0.25
VTH = 0.3
N_CORES = 8
P = 128
T = 8
B_LOC = 4  # batches per core
PIX_PER_CORE = B_LOC * 128 * 32 * 32  # 524288
NPP = PIX_PER_CORE // P  # 4096 pixels per partition

_CACHE = {}

# Config key (A/B-tested on hardware):
#   h<k>  : planes 0/1 interleave chunk size k
#   sr<n> : second halves of planes 2..n+1 ride the Scalar ring
CFG = "v12_h2048_sr0"


def _parse(key):
    h, sr = 1024, 3
    for tok in key.split("_"):
        if tok.startswith("h") and tok[1:].isdigit():
            h = int(tok[1:])
        if tok.startswith("sr") and tok[2:].isdigit():
            sr = int(tok[2:])
    return dict(h=h, sr=sr, noaeb="noaeb" in key)


def _build_nc(key=None):
    if key is None:
        key = CFG
    cfg = _parse(key)
    import concourse.tile as tile
    from concourse import bacc, mybir

    f32 = mybir.dt.float32
    i8 = mybir.dt.int8
    Alu = mybir.AluOpType
    AF = mybir.ActivationFunctionType

    nc = bacc.Bacc(
        "TRN2",
        target_bir_lowering=False,
        debug=False,
        enable_asserts=False,
        num_devices=N_CORES,
    )
    x_d = nc.dram_tensor("x", [P, T, NPP], f32, kind="ExternalInput").ap()
    o_d = nc.dram_tensor("o", [P, T, NPP], i8, kind="ExternalOutput").ap()

    # ACT activation bias needs a pre-registered const AP.
    cb = nc.alloc_sbuf_tensor("const-f32-negvth", [128, 1], f32)
    nc.gpsimd.memset(cb.ap(), -VTH)
    nc.const_aps.aps[(f32, -VTH)] = cb.ap()
    if cfg["noaeb"]:
        nc.multi_engine_barrier(
            [mybir.EngineType.Pool, mybir.EngineType.Activation]
        )
    else:
        nc.all_engine_barrier()

    H = cfg["h"]
    SR = cfg["sr"]  # planes 2..SR+1 second halves on scalar ring

    with tile.TileContext(nc) as tc:
        with tc.tile_pool(name="pp", bufs=1) as pp:
            xt = pp.tile([P, T, NPP], f32, tag="xt")
            c = pp.tile([P, NPP], f32, tag="c")
            o8 = pp.tile([P, T, NPP], i8, tag="o8")

            # Input: planes 0/1 interleaved in H-column chunks on Sync.
            bounds = list(range(0, NPP, H)) + [NPP]
            chunks = list(zip(bounds[:-1], bounds[1:]))
            for lo, hi in chunks:
                nc.sync.dma_start(xt[:, 0, lo:hi], x_d[:, 0, lo:hi])
                nc.sync.dma_start(xt[:, 1, lo:hi], x_d[:, 1, lo:hi])
            # Planes 2..SR+1: first half Sync, second half Scalar ring
            # (enqueued before any output DMA so they clear the ring first).
            for t in range(2, T):
                if 2 <= t < 2 + SR:
                    nc.sync.dma_start(xt[:, t, :2048], x_d[:, t, :2048])
                    nc.scalar.dma_start(xt[:, t, 2048:], x_d[:, t, 2048:])
                else:
                    nc.sync.dma_start(xt[:, t, :], x_d[:, t, :])

            def cu(t, sl):
                up = xt[:, t - 1, sl]
                nc.vector.scalar_tensor_tensor(
                    c[:, sl], up, VTH, up, op0=Alu.is_le, op1=Alu.mult
                )
                nc.vector.scalar_tensor_tensor(
                    xt[:, t, sl], c[:, sl], TAU, xt[:, t, sl],
                    op0=Alu.mult, op1=Alu.add,
                )

            # Plane 0: u_0 = x_0, spike immediately.
            nc.scalar.activation(o8[:, 0, :], xt[:, 0, :], AF.Sign, bias=-VTH)
            nc.scalar.dma_start(o_d[:, 0, :], o8[:, 0, :])

            # Step 1: chunked (chases the chunked DMAs).
            for lo, hi in chunks:
                cu(1, slice(lo, hi))
            nc.scalar.activation(o8[:, 1, :], xt[:, 1, :], AF.Sign, bias=-VTH)
            nc.scalar.dma_start(o_d[:, 1, :], o8[:, 1, :])

            # Steps 2..T-2: full-plane ops (minimal op count).
            for t in range(2, T - 1):
                cu(t, slice(0, NPP))
                nc.scalar.activation(o8[:, t, :], xt[:, t, :], AF.Sign, bias=-VTH)
                nc.scalar.dma_start(o_d[:, t, :], o8[:, t, :])

            # Step T-1: quartered with interleaved signs/outs (short tail).
            n7 = 4
            C7 = NPP // n7
            for q in range(n7):
                sl = slice(q * C7, (q + 1) * C7)
                cu(T - 1, sl)
                if q == n7 - 1:
                    # Last quarter's spike on the (now otherwise-done) DVE:
                    # is_gt -> int8 {0,1}; host maps >0 so both encodings work.
                    nc.vector.tensor_scalar(
                        o8[:, T - 1, sl], xt[:, T - 1, sl], VTH, None,
                        op0=Alu.is_gt,
                    )
                else:
                    nc.scalar.activation(
                        o8[:, T - 1, sl], xt[:, T - 1, sl], AF.Sign, bias=-VTH
                    )
                nc.scalar.dma_start(o_d[:, T - 1, sl], o8[:, T - 1, sl])
    nc.compile()
    return nc


def _get_nc(key=None):
    k = key or CFG
    if k not in _CACHE:
        _CACHE[k] = _build_nc(k)
    return _CACHE[k]


def _shard(x: np.ndarray):
    xs = np.ascontiguousarray(x, dtype=np.float32)
    return [
        np.ascontiguousarray(
            xs[i * B_LOC : (i + 1) * B_LOC].reshape(P, NPP, T).transpose(0, 2, 1)
        )
        for i in range(N_CORES)
    ]


def _run(in_maps, key=None, **kwargs):
    from concourse.bass_utils import run_bass_kernel_spmd

    nc = _get_nc(key)
    return run_bass_kernel_spmd(nc, in_maps, core_ids=list(range(N_CORES)), **kwargs)


def kernel(x: np.ndarray) -> np.ndarray:
    in_maps = [{"x": s} for s in _shard(x)]
    res = _run(in_maps)
    outs = []
    for i in range(N_CORES):
        s8 = res.results[i]["o"]  # [P, T, NPP] int8 sign values
        o = (s8 > 0).transpose(0, 2, 1).astype(np.float32)  # [P, NPP, T]
        outs.append(o.reshape(B_LOC, 128, 32, 32, T))
    return np.concatenate(outs, axis=0)
